# revision 34
# baseline (speedup 1.0000x reference)
"""3-layer GCN (message passing) on 8 NeuronCores via Bass/Tile.

Strategy (vertex-cut / dst-sharding, bf16 data path):
  - Nodes are LPT-packed into (core, block) bins by in-degree so every
    128-dst block needs a similar number of edge chunks on every core
    (SPMD program). Output rows are un-permuted on the host at the end.
  - Self loops are regular edges (src=dst), no special casing.
  - Layer 1 is re-associated: relu(A_hat (X W1) + b1) = relu((A_hat X') W1
    + b1) with X' = dinv*X pre-scaled on host and expanded to edge order
    (xe), so L1 needs no indirect gathers at all - pure streaming.
  - Layers 2/3: each core computes its Y = dinv*(h @ W) shard; the table
    is AllGathered in FOUR row-pieces (separate DRAM tensors so the tile
    dep tracker gives piece-granular ordering), interleaved with the
    producing loop. Edges are grouped by (7-block group, source piece)
    and fetched with batched gpsimd dma_gathers of <=1024 rows (=64
    descriptors/SDMA engine, the packet ceiling; larger launches wedge
    the device). Measured SWDGE cost ~1us fixed + ~7.4ns/row of Q7
    descriptor generation - that per-row cost is the kernel's floor.
  - Scatter-add realized on TensorE with one-hot scatter matrices
    S[e, dst] = dinv[dst] built ON-CHIP per BLOCK by two broadcast
    VectorE tensor_tensors: (iota == dstcol bcast) * dinvd bcast
    (per-chunk tensor_scalar was ~716ns/op on DVE and jammed it).
  - L1/L2 scatter runs transposed (psT = G^T @ S) so the ReLU epilogue
    writes h^T directly into the resident xts tile; bias+relu fused into
    one ScalarE activation per feature half. The next layer's phase-1
    window is interleaved after each block.
  - L3 scatter runs direct (ps = S^T @ G) to emit [dst, feat] fp32 rows.
"""

import os
import sys

sys.path.insert(0, "/opt/trn_rl_repo")

import numpy as np
import ml_dtypes

BF16 = ml_dtypes.bfloat16

N = 50000
E = 500000
NC = 8
SH = N // NC            # 6250 nodes per core
P = 128
DIN = 128
DH = 256
NBLK = (SH + P - 1) // P      # 49 dst blocks per core
LASTM = SH - (NBLK - 1) * P   # 106 dsts in the last block
GSZ = 7                       # blocks per gather group
NG = NBLK // GSZ              # 7 groups
NQ = 4                        # AllGather pieces
AG_WB = [0, 13, 25, 37, NBLK]            # window boundaries per AG piece
AG_LO = [w * P for w in AG_WB[:NQ]]       # piece start rows (per core)
RQ = [min(AG_WB[q + 1] * P, SH) - AG_LO[q] for q in range(NQ)]  # piece rows


def _balance(deg):
    """LPT-pack nodes into (core, block) bins by in-edge weight so every
    block's edge count is ~equal across cores. Returns perm: node -> device
    row (core*SH + block*128 + slot)."""
    import heapq
    w = deg.astype(np.int64)
    nodes = np.argsort(-w, kind="stable")
    caps = []
    for c in range(NC):
        for b in range(NBLK):
            cap = LASTM if b == NBLK - 1 else P
            caps.append((c * SH + b * P, cap))
    heap = [(0, i) for i in range(len(caps))]
    heapq.heapify(heap)
    fill = [0] * len(caps)
    perm = np.empty(N, dtype=np.int64)
    for n in nodes:
        while True:
            wt, i = heapq.heappop(heap)
            if fill[i] < caps[i][1]:
                break
        perm[n] = caps[i][0] + fill[i]
        fill[i] += 1
        if fill[i] < caps[i][1]:
            heapq.heappush(heap, (wt + int(w[n]), i))
    return perm


def _layout():
    """Static (data-independent) meta helpers."""
    pass


def _preprocess(x, edge_index):
    """Host-side graph partitioning. Returns per-core tensors + layout meta."""
    src = np.asarray(edge_index[0], dtype=np.int64)
    dst = np.asarray(edge_index[1], dtype=np.int64)
    deg = np.bincount(dst, minlength=N).astype(np.float64)
    dinv0 = (1.0 / np.sqrt(deg + 1.0)).astype(np.float32)

    perm = _balance(deg)
    inv = np.empty(N, dtype=np.int64)
    inv[perm] = np.arange(N)           # device row -> original node
    src = perm[src]
    dst = perm[dst]
    x = np.asarray(x, np.float32)[inv]
    dinv = dinv0[inv]

    # self loops as regular edges
    loop = np.arange(N, dtype=np.int64)
    src = np.concatenate([src, loop])
    dst = np.concatenate([dst, loop])

    order = np.argsort(dst, kind="stable")
    s_s = src[order]
    d_s = dst[order]
    bounds = np.searchsorted(d_s, np.arange(NC + 1) * SH)

    xs = (dinv[:, None] * x).astype(BF16)  # [N, 128] pre-scaled features

    # source piece + piece-local table row for every edge
    gc = s_s // SH
    gr = s_s % SH
    q_of = np.searchsorted(np.array(AG_LO[1:]), gr, side="right")  # [TE]
    rq = np.array(RQ)[q_of]
    lo = np.array(AG_LO)[q_of]
    prow = gc * rq + (gr - lo)         # row within piece-q gathered table
    blk = np.minimum((d_s % SH) >> 7, NBLK - 1)

    # per-core per-(block, piece) counts -> shared chunk counts k2
    cnt = np.zeros((NC, NBLK, NQ), dtype=np.int64)
    for c in range(NC):
        lo_, hi_ = bounds[c], bounds[c + 1]
        np.add.at(cnt[c], (blk[lo_:hi_], q_of[lo_:hi_]), 1)
    k2 = (cnt.max(axis=0) + P - 1) // P            # [NBLK, NQ]
    cnt1 = cnt.sum(axis=2)                         # [NC, NBLK]
    k1 = np.maximum(1, (cnt1.max(axis=0) + P - 1) // P)  # [NBLK]
    cum1 = np.concatenate([[0], np.cumsum(k1)]).astype(int)
    nch1 = int(cum1[-1])

    # L2/L3 column layout: group-major, then piece, then block
    colstart = np.zeros((NBLK, NQ), dtype=np.int64)
    cstart = np.zeros((NG, NQ), dtype=np.int64)
    K = np.zeros((NG, NQ), dtype=np.int64)
    gstart = np.zeros(NG + 1, dtype=np.int64)
    pos = 0
    for g in range(NG):
        gstart[g] = pos
        for q in range(NQ):
            cstart[g, q] = pos
            for b in range(g * GSZ, (g + 1) * GSZ):
                colstart[b, q] = pos
                pos += k2[b, q]
            K[g, q] = pos - cstart[g, q]
    gstart[NG] = pos
    nch2 = int(pos)
    cols2 = [
        [int(colstart[b, q]) + i for q in range(NQ) for i in range(k2[b, q])]
        for b in range(NBLK)
    ]
    # block-major S-table column layout (for contiguous per-block S build)
    cp2 = k2.sum(axis=1)                        # chunks per block
    scum2 = np.concatenate([[0], np.cumsum(cp2)]).astype(int)
    # global gt column -> block-major S column
    s_of_col = np.zeros(nch2, dtype=np.int64)
    for b in range(NBLK):
        for ci, col in enumerate(cols2[b]):
            s_of_col[col] = scum2[b] + ci
    wstart = np.zeros((NG, NQ), dtype=np.int64)
    tw = 0
    for g in range(NG):
        for q in range(NQ):
            wstart[g, q] = tw
            tw += 8 * int(K[g, q])
    TW = int(tw)

    per_core = []
    for c in range(NC):
        lo_, hi_ = bounds[c], bounds[c + 1]
        b_c = blk[lo_:hi_]
        q_c = q_of[lo_:hi_]
        pr_c = prow[lo_:hi_]
        d_c = d_s[lo_:hi_] - c * SH
        s_c = s_s[lo_:hi_]

        # ---- L2/L3 slot assignment: order by (block, piece) ----
        key = b_c * NQ + q_c
        o2 = np.argsort(key, kind="stable")
        key_s = key[o2]
        # position within each (b,q) run
        starts = np.searchsorted(key_s, np.arange(NBLK * NQ))
        pos_in = np.arange(len(key_s)) - starts[key_s]
        chunk = pos_in >> 7
        part = pos_in & 127
        col = colstart.reshape(-1)[key_s] + chunk     # global chunk col

        scol = s_of_col[col]                    # block-major S column
        dstcol2 = np.full((P, nch2), -1, dtype=np.int16)
        dinvd2 = np.zeros((P, nch2), dtype=np.float32)  # cast to bf16 below
        dloc = d_c[o2]
        dstcol2[part, scol] = (dloc & 127).astype(np.int16)
        dinvd2[part, scol] = dinv[dloc + c * SH]

        # gather index table: wrapped-16 i16 layout per (g, q) region
        big = np.zeros(P * nch2, dtype=np.int16)
        big[col * P + part] = pr_c[o2].astype(np.int16)
        idx2 = np.zeros((P, TW), dtype=np.int16)
        for g in range(NG):
            for q in range(NQ):
                kk = int(K[g, q])
                if kk == 0:
                    continue
                arr = big[int(cstart[g, q]) * P:(int(cstart[g, q]) + kk) * P]
                wr = arr.reshape(-1, 16).T            # [16, 8*kk]
                ws = int(wstart[g, q])
                idx2[:, ws:ws + 8 * kk] = np.tile(wr, (8, 1))

        # ---- L1 slot assignment: order by block only ----
        o1 = np.argsort(b_c, kind="stable")
        b1s = b_c[o1]
        starts1 = np.searchsorted(b1s, np.arange(NBLK))
        pos1 = np.arange(len(b1s)) - starts1[b1s]
        chunk_1 = pos1 >> 7
        part1 = pos1 & 127
        col1 = cum1[b1s] + chunk_1

        xe = np.zeros((P, nch1, DIN), dtype=BF16)
        xe[part1, col1, :] = xs[s_c[o1]]
        d1 = d_c[o1]
        # L1 scatter matrices prebuilt on host and streamed (HBM is idle
        # during L1; keeps the serial L1 prefix off the DVE)
        sall1 = np.zeros((P, nch1, P), dtype=BF16)
        sall1[part1, col1, d1 & 127] = dinv[d1 + c * SH]
        sall1 = sall1.reshape(P, nch1 * P)
        xe = xe.reshape(P, nch1 * DIN)

        # dinv of own shard in [p, w] window layout (phase-1 scaling)
        ids = c * SH + np.arange(NBLK * P)
        valid = ids < (c + 1) * SH
        dc_own = np.where(valid, dinv[np.minimum(ids, N - 1)], 0.0)
        dc_own = dc_own.reshape(NBLK, P).T.astype(np.float32).copy()

        per_core.append({
            "xe": xe,
            "sal1": sall1,
            "dc2": dstcol2, "dv2": dinvd2.astype(BF16),
            "idx2": idx2,
            "dco": dc_own,
        })

    meta = {
        "k2": k2.tolist(), "k1": k1.tolist(), "cum1": cum1.tolist(),
        "nch1": nch1, "nch2": nch2, "TW": TW,
        "cstart": cstart.tolist(), "K": K.tolist(),
        "gstart": gstart.tolist(), "wstart": wstart.tolist(),
        "cols2": cols2, "cp2": cp2.tolist(), "scum2": scum2.tolist(),
    }
    return per_core, meta, perm


def _build_program(meta):
    from concourse import bass, bacc, mybir
    import concourse.tile as tile

    f32 = mybir.dt.float32
    bf16 = mybir.dt.bfloat16
    i16 = mybir.dt.int16
    i32 = mybir.dt.int32
    k1 = meta["k1"]
    cum1 = meta["cum1"]
    nch1 = meta["nch1"]
    nch2 = meta["nch2"]
    TW = meta["TW"]
    wstart = meta["wstart"]
    cstart = meta["cstart"]
    K = meta["K"]
    gstart = meta["gstart"]
    cols2 = meta["cols2"]
    cp2 = meta["cp2"]
    scum2 = meta["scum2"]
    mxk1 = max(k1)
    mxcp = max(max(k1), max(cp2))
    mxgw = max(gstart[g + 1] - gstart[g] for g in range(NG))

    nc = bacc.Bacc("TRN2", target_bir_lowering=False, debug=False,
                   dynamic_dma_scratch_size=65536)

    xe = nc.declare_dram_parameter("xe", [P, nch1 * DIN], bf16, isOutput=False)
    sal1 = nc.declare_dram_parameter("sal1", [P, nch1 * P], bf16,
                                     isOutput=False)
    dc2 = nc.declare_dram_parameter("dc2", [P, nch2], i16, isOutput=False)
    dv2 = nc.declare_dram_parameter("dv2", [P, nch2], bf16, isOutput=False)
    idx2 = nc.declare_dram_parameter("idx2", [P, TW], i16, isOutput=False)
    dco = nc.declare_dram_parameter("dco", [P, NBLK], f32, isOutput=False)
    w1 = nc.declare_dram_parameter("w1", [P, DH], bf16, isOutput=False)
    w2p = nc.declare_dram_parameter("w2p", [P, 2 * DH], bf16, isOutput=False)
    w3p = nc.declare_dram_parameter("w3p", [P, 2 * DH], bf16, isOutput=False)
    bt = nc.declare_dram_parameter("bt", [P, 4], f32, isOutput=False)
    bf3 = nc.declare_dram_parameter("bf3", [P, DH], f32, isOutput=False)
    iot = nc.declare_dram_parameter("iot", [P, mxcp * P], i16, isOutput=False)
    outp = nc.declare_dram_parameter("out", [SH, DH], f32, isOutput=True)

    ybin2 = [nc.dram_tensor(f"ybin2_{q}", [RQ[q], DH], bf16) for q in range(NQ)]
    ybout2 = [nc.dram_tensor(f"ybout2_{q}", [NC * RQ[q], DH], bf16,
                             addr_space="Shared") for q in range(NQ)]
    ybin3 = [nc.dram_tensor(f"ybin3_{q}", [RQ[q], DH], bf16) for q in range(NQ)]
    ybout3 = [nc.dram_tensor(f"ybout3_{q}", [NC * RQ[q], DH], bf16,
                             addr_space="Shared") for q in range(NQ)]

    AG = mybir.AluOpType
    ACT = mybir.ActivationFunctionType

    def piece_of_window(w):
        for q in range(NQ):
            if AG_WB[q] <= w < AG_WB[q + 1]:
                return q
        raise AssertionError

    with tile.TileContext(nc, linearize=bool(os.environ.get("KLIN"))) as tc:
        with (
            tc.tile_pool(name="const", bufs=1) as cp_,
            tc.tile_pool(name="sb", bufs=2) as sb,
            tc.tile_pool(name="stp", bufs=2) as stp,
            tc.tile_pool(name="gp", bufs=2) as gp,
            tc.tile_pool(name="xb", bufs=2) as xbp,
            tc.tile_pool(name="pp", bufs=2, space="PSUM") as pp,
            tc.tile_pool(name="ph", bufs=6, space="PSUM") as ph,
        ):
            w1sb = cp_.tile([P, DH], dtype=bf16)
            nc.sync.dma_start(out=w1sb[:], in_=w1[:, :])
            w2sb = cp_.tile([P, 2 * DH], dtype=bf16)
            nc.sync.dma_start(out=w2sb[:], in_=w2p[:, :])
            w3sb = cp_.tile([P, 2 * DH], dtype=bf16)
            nc.sync.dma_start(out=w3sb[:], in_=w3p[:, :])
            btsb = cp_.tile([P, 4], dtype=f32)
            nc.sync.dma_start(out=btsb[:], in_=bt[:, :])
            bf3sb = cp_.tile([P, DH], dtype=f32)
            nc.sync.dma_start(out=bf3sb[:], in_=bf3[:, :])
            dcosb = cp_.tile([P, NBLK], dtype=f32)
            nc.sync.dma_start(out=dcosb[:], in_=dco[:, :])
            iotsb = cp_.tile([P, mxcp * P], dtype=i16)
            nc.sync.dma_start(out=iotsb[:], in_=iot[:, :])
            dc2sb = cp_.tile([P, nch2], dtype=i16)
            nc.sync.dma_start(out=dc2sb[:], in_=dc2[:, :])
            dv2sb = cp_.tile([P, nch2], dtype=bf16)
            nc.sync.dma_start(out=dv2sb[:], in_=dv2[:, :])
            idxsb = cp_.tile([P, TW], dtype=i16)
            nc.sync.dma_start(out=idxsb[:], in_=idx2[:, :])
            # resident transposed activations h^T: half h at cols [h*SH, ...)
            xts = cp_.tile([P, 2 * SH], dtype=bf16)

            def sbuild_block(dcsb, dvsb, c0, cp):
                """All of a block's one-hot scatter chunks in two DVE ops:
                S[p, k*128+c] = (c == dstcol[p, c0+k]) * dinv[p, c0+k]."""
                eq = stp.tile([P, mxcp * P], dtype=bf16, tag="eq")
                nc.vector.tensor_tensor(
                    out=eq[:, :cp * P].rearrange("p (k e) -> p k e", e=P),
                    in0=iotsb[:, :cp * P].rearrange("p (k e) -> p k e", e=P),
                    in1=dcsb[:, c0:c0 + cp].to_broadcast((P, cp, P)),
                    op=AG.is_equal)
                st = stp.tile([P, mxcp * P], dtype=bf16, tag="st")
                nc.vector.tensor_tensor(
                    out=st[:, :cp * P].rearrange("p (k e) -> p k e", e=P),
                    in0=eq[:, :cp * P].rearrange("p (k e) -> p k e", e=P),
                    in1=dvsb[:, c0:c0 + cp].to_broadcast((P, cp, P)),
                    op=AG.mult)
                return st

            def phase1_win(wsb, ybinq, w):
                """One window of Y = dinv * (h @ W) from xts -> ybin rows."""
                m = LASTM if w == NBLK - 1 else P
                ps = pp.tile([P, DH], dtype=f32, tag="ps")
                for h in range(2):
                    nc.tensor.matmul(
                        out=ps[:m, :],
                        lhsT=xts[:, h * SH + w * P:h * SH + w * P + m],
                        rhs=wsb[:, h * DH:(h + 1) * DH],
                        start=(h == 0), stop=(h == 1))
                ysb = sb.tile([P, DH], dtype=bf16, tag="ysb")
                nc.scalar.activation(out=ysb[:m, :], in_=ps[:m, :],
                                     func=ACT.Copy,
                                     scale=dcosb[:m, w:w + 1])
                q = piece_of_window(w)
                r0 = w * P - AG_LO[q]
                nc.sync.dma_start(out=ybinq[q][r0:r0 + m, :], in_=ysb[:m, :])

            def all_gather_piece(ybinq, yboutq, q):
                nc.gpsimd.collective_compute(
                    "AllGather", AG.bypass,
                    replica_groups=[list(range(NC))],
                    ins=[ybinq[q][0:RQ[q], :].opt()],
                    outs=[yboutq[q][0:NC * RQ[q], :].opt()])

            gmax = int(os.environ.get("KGMAX", "8"))   # chunks per gather

            def gather_pieces(gt, g, tableq, pieces):
                """Issue <=gmax-chunk dma_gathers for the given pieces of
                group g into tile gt (the wrapped-16 idx layout slices at
                chunk granularity). NOTE: the gpsimd queue is in-order, so a
                gather must be issued AFTER its piece's AllGather or the
                queue-head wait deadlocks."""
                g0 = gstart[g]
                for q in pieces:
                    kk = K[g][q]
                    c0 = cstart[g][q] - g0      # column offset inside tile
                    ws = wstart[g][q]
                    for j0 in range(0, kk, gmax):
                        j1 = min(j0 + gmax, kk)
                        out_ap = gt[:, (c0 + j0) * DH:(c0 + j1) * DH].rearrange(
                            "p (k e) -> p k e", e=DH)
                        nc.gpsimd.dma_gather(
                            out_ap,
                            tableq[q][0:NC * RQ[q], :],
                            idxsb[:, ws + 8 * j0:ws + 8 * j1],
                            P * (j1 - j0),
                            P * (j1 - j0),
                            DH)
                return gt

            NHOIST = 2          # groups whose early-piece gathers are issued
                                # before the last AG piece, so the Pool engine
                                # works instead of idling behind it

            def hoist_gathers(tableq):
                """Pre-issue pieces 0..NQ-2 for the first NHOIST groups."""
                pre = {}
                for g in range(NHOIST):
                    gt = gp.tile([P, mxgw * DH], dtype=bf16, tag="gt")
                    gather_pieces(gt, g, tableq, list(range(NQ - 1)))
                    pre[g] = gt
                return pre

            def group_tile(pre, g, tableq):
                if g in pre:
                    gt = pre.pop(g)
                    gather_pieces(gt, g, tableq, [NQ - 1])
                else:
                    gt = gp.tile([P, mxgw * DH], dtype=bf16, tag="gt")
                    gather_pieces(gt, g, tableq, list(range(NQ)))
                return gt

            # ---------------- Layer 1: streamed edge table ------------------
            for b in range(NBLK):
                kb = k1[b]
                m = LASTM if b == NBLK - 1 else P
                xet = xbp.tile([P, mxk1 * DIN], dtype=bf16, tag="xet")
                nc.sync.dma_start(
                    out=xet[:, :kb * DIN],
                    in_=xe[:, cum1[b] * DIN:(cum1[b] + kb) * DIN])
                psa = ph.tile([P, P], dtype=f32, tag="half")
                stb = stp.tile([P, mxcp * P], dtype=bf16, tag="st")
                nc.sync.dma_start(
                    out=stb[:, :kb * P],
                    in_=sal1[:, cum1[b] * P:(cum1[b] + kb) * P])
                for i in range(kb):
                    nc.tensor.matmul(
                        out=psa[:, :m],
                        lhsT=xet[:, i * DIN:(i + 1) * DIN],
                        rhs=stb[:, i * P:i * P + m],
                        start=(i == 0), stop=(i == kb - 1))
                agg = sb.tile([P, P], dtype=bf16, tag="agg")
                nc.scalar.activation(out=agg[:, :m], in_=psa[:, :m],
                                     func=ACT.Copy)
                psb = [ph.tile([P, P], dtype=f32, tag="half", name=f"psb{h}")
                       for h in range(2)]
                for h in range(2):
                    nc.tensor.matmul(
                        out=psb[h][:, :m],
                        lhsT=w1sb[:, h * P:(h + 1) * P],
                        rhs=agg[:, :m],
                        start=True, stop=True)
                for h in range(2):
                    nc.scalar.activation(
                        out=xts[:, h * SH + b * P:h * SH + b * P + m],
                        in_=psb[h][:, :m],
                        func=ACT.Relu, bias=btsb[:, h:h + 1])
                phase1_win(w2sb, ybin2, b)
                if b + 1 in AG_WB[1:NQ]:
                    all_gather_piece(ybin2, ybout2, AG_WB.index(b + 1) - 1)
            pre2 = hoist_gathers(ybout2)
            all_gather_piece(ybin2, ybout2, NQ - 1)

            # ---------------- Layer 2: transposed scatter -------------------
            for g in range(NG):
                gt = group_tile(pre2, g, ybout2)
                g0 = gstart[g]
                for b in range(g * GSZ, (g + 1) * GSZ):
                    m = LASTM if b == NBLK - 1 else P
                    cols = cols2[b]
                    pst = [ph.tile([P, P], dtype=f32, tag="half",
                                   name=f"pst{h}") for h in range(2)]
                    stb = sbuild_block(dc2sb, dv2sb, scum2[b], len(cols))
                    for ci, col in enumerate(cols):
                        lp = col - g0
                        for h in range(2):
                            nc.tensor.matmul(
                                out=pst[h][:, :m],
                                lhsT=gt[:, lp * DH + h * P:lp * DH + (h + 1) * P],
                                rhs=stb[:, ci * P:ci * P + m],
                                start=(ci == 0), stop=(ci == len(cols) - 1))
                    for h in range(2):
                        nc.scalar.activation(
                            out=xts[:, h * SH + b * P:h * SH + b * P + m],
                            in_=pst[h][:, :m],
                            func=ACT.Relu, bias=btsb[:, 2 + h:2 + h + 1])
                    phase1_win(w3sb, ybin3, b)
                    if b + 1 in AG_WB[1:NQ]:
                        all_gather_piece(ybin3, ybout3, AG_WB.index(b + 1) - 1)
            pre3 = hoist_gathers(ybout3)
            all_gather_piece(ybin3, ybout3, NQ - 1)

            # ---------------- Layer 3: direct scatter -> out ----------------
            for g in range(NG):
                gt = group_tile(pre3, g, ybout3)
                g0 = gstart[g]
                for b in range(g * GSZ, (g + 1) * GSZ):
                    m = LASTM if b == NBLK - 1 else P
                    cols = cols2[b]
                    ps3 = pp.tile([P, DH], dtype=f32, tag="ps")
                    stb = sbuild_block(dc2sb, dv2sb, scum2[b], len(cols))
                    for ci, col in enumerate(cols):
                        lp = col - g0
                        nc.tensor.matmul(
                            out=ps3[:m, :],
                            lhsT=stb[:, ci * P:ci * P + m],
                            rhs=gt[:, lp * DH:(lp + 1) * DH],
                            start=(ci == 0), stop=(ci == len(cols) - 1))
                    osb = sb.tile([P, DH], dtype=f32, tag="osb")
                    nc.vector.tensor_tensor(out=osb[:m, :], in0=ps3[:m, :],
                                            in1=bf3sb[:m, :], op=AG.add)
                    nc.sync.dma_start(out=outp[b * P:b * P + m, :],
                                      in_=osb[:m, :])

    nc.compile()
    return nc


def kernel(x, edge_index, W1, b1, W2, b2, W3, b3, _trace=False):
    from concourse.bass_utils import run_bass_kernel_spmd

    x = np.asarray(x, dtype=np.float32)
    per_core, meta, perm = _preprocess(x, edge_index)
    nc = _build_program(meta)

    w2 = np.asarray(W2, np.float32)
    w3 = np.asarray(W3, np.float32)
    w2p = np.concatenate([w2[0:P, :], w2[P:2 * P, :]], axis=1).astype(BF16)
    w3p = np.concatenate([w3[0:P, :], w3[P:2 * P, :]], axis=1).astype(BF16)
    b1v = np.asarray(b1, np.float32)
    b2v = np.asarray(b2, np.float32)
    bt = np.stack([b1v[0:P], b1v[P:2 * P], b2v[0:P], b2v[P:2 * P]],
                  axis=1).astype(np.float32)
    common = {
        "w1": np.asarray(W1, np.float32).astype(BF16),
        "w2p": w2p,
        "w3p": w3p,
        "bt": bt,
        "bf3": np.broadcast_to(np.asarray(b3, np.float32), (P, DH)).copy(),
    }
    mxcp = max(max(meta["k1"]), max(meta["cp2"]))
    common["iot"] = np.broadcast_to(
        np.tile(np.arange(P, dtype=np.int16), mxcp), (P, mxcp * P)).copy()
    in_maps = []
    for c in range(NC):
        m = dict(common)
        m.update(per_core[c])
        in_maps.append(m)

    res = run_bass_kernel_spmd(nc, in_maps, list(range(NC)), trace=_trace)
    shards = [res.results[c]["out"] for c in range(NC)]
    out = np.concatenate(shards, axis=0)[perm]
    if _trace:
        return out, res
    return out


# revision 35
# speedup vs baseline: 1.0281x; 1.0281x over previous
"""3-layer GCN (message passing) on 8 NeuronCores via Bass/Tile.

Strategy (vertex-cut / dst-sharding, bf16 data path):
  - Nodes are LPT-packed into (core, block) bins by in-degree so every
    128-dst block needs a similar number of edge chunks on every core
    (SPMD program). Output rows are un-permuted on the host at the end.
  - Self loops are regular edges (src=dst), no special casing.
  - Layer 1 is re-associated: relu(A_hat (X W1) + b1) = relu((A_hat X') W1
    + b1) with X' = dinv*X pre-scaled on host and expanded to edge order
    (xe), so L1 needs no indirect gathers at all - pure streaming.
  - Layers 2/3: each core computes its Y = dinv*(h @ W) shard; the table
    is AllGathered in FOUR row-pieces (separate DRAM tensors so the tile
    dep tracker gives piece-granular ordering), interleaved with the
    producing loop. Edges are grouped by (7-block group, source piece)
    and fetched with batched gpsimd dma_gathers of <=1024 rows (=64
    descriptors/SDMA engine, the packet ceiling; larger launches wedge
    the device). Measured SWDGE cost ~1us fixed + ~7.4ns/row of Q7
    descriptor generation - that per-row cost is the kernel's floor.
  - Scatter-add realized on TensorE with one-hot scatter matrices
    S[e, dst] = dinv[dst] built ON-CHIP per BLOCK by two broadcast
    VectorE tensor_tensors: (iota == dstcol bcast) * dinvd bcast
    (per-chunk tensor_scalar was ~716ns/op on DVE and jammed it).
  - L1/L2 scatter runs transposed (psT = G^T @ S) so the ReLU epilogue
    writes h^T directly into the resident xts tile; bias+relu fused into
    one ScalarE activation per feature half. The next layer's phase-1
    window is interleaved after each block.
  - L3 scatter runs direct (ps = S^T @ G) to emit [dst, feat] fp32 rows.
"""

import os
import sys

sys.path.insert(0, "/opt/trn_rl_repo")

import numpy as np
import ml_dtypes

BF16 = ml_dtypes.bfloat16

N = 50000
E = 500000
NC = 8
SH = N // NC            # 6250 nodes per core
P = 128
DIN = 128
DH = 256
NBLK = (SH + P - 1) // P      # 49 dst blocks per core
LASTM = SH - (NBLK - 1) * P   # 106 dsts in the last block
GSZ = 7                       # blocks per gather group
NG = NBLK // GSZ              # 7 groups
NQ = 4                        # AllGather pieces
AG_WB = [0, 13, 25, 37, NBLK]            # window boundaries per AG piece
AG_LO = [w * P for w in AG_WB[:NQ]]       # piece start rows (per core)
RQ = [min(AG_WB[q + 1] * P, SH) - AG_LO[q] for q in range(NQ)]  # piece rows


def _balance(deg):
    """LPT-pack nodes into (core, block) bins by in-edge weight so every
    block's edge count is ~equal across cores. Returns perm: node -> device
    row (core*SH + block*128 + slot)."""
    import heapq
    w = deg.astype(np.int64)
    nodes = np.argsort(-w, kind="stable")
    caps = []
    for c in range(NC):
        for b in range(NBLK):
            cap = LASTM if b == NBLK - 1 else P
            caps.append((c * SH + b * P, cap))
    heap = [(0, i) for i in range(len(caps))]
    heapq.heapify(heap)
    fill = [0] * len(caps)
    perm = np.empty(N, dtype=np.int64)
    for n in nodes:
        while True:
            wt, i = heapq.heappop(heap)
            if fill[i] < caps[i][1]:
                break
        perm[n] = caps[i][0] + fill[i]
        fill[i] += 1
        if fill[i] < caps[i][1]:
            heapq.heappush(heap, (wt + int(w[n]), i))
    return perm


def _layout():
    """Static (data-independent) meta helpers."""
    pass


def _preprocess(x, edge_index):
    """Host-side graph partitioning. Returns per-core tensors + layout meta."""
    src = np.asarray(edge_index[0], dtype=np.int64)
    dst = np.asarray(edge_index[1], dtype=np.int64)
    deg = np.bincount(dst, minlength=N).astype(np.float64)
    dinv0 = (1.0 / np.sqrt(deg + 1.0)).astype(np.float32)

    perm = _balance(deg)
    inv = np.empty(N, dtype=np.int64)
    inv[perm] = np.arange(N)           # device row -> original node
    src = perm[src]
    dst = perm[dst]
    x = np.asarray(x, np.float32)[inv]
    dinv = dinv0[inv]

    # self loops as regular edges
    loop = np.arange(N, dtype=np.int64)
    src = np.concatenate([src, loop])
    dst = np.concatenate([dst, loop])

    order = np.argsort(dst, kind="stable")
    s_s = src[order]
    d_s = dst[order]
    bounds = np.searchsorted(d_s, np.arange(NC + 1) * SH)

    xs = (dinv[:, None] * x).astype(BF16)  # [N, 128] pre-scaled features

    # source piece + piece-local table row for every edge
    gc = s_s // SH
    gr = s_s % SH
    q_of = np.searchsorted(np.array(AG_LO[1:]), gr, side="right")  # [TE]
    rq = np.array(RQ)[q_of]
    lo = np.array(AG_LO)[q_of]
    prow = gc * rq + (gr - lo)         # row within piece-q gathered table
    blk = np.minimum((d_s % SH) >> 7, NBLK - 1)

    # per-core per-(block, piece) counts -> shared chunk counts k2
    cnt = np.zeros((NC, NBLK, NQ), dtype=np.int64)
    for c in range(NC):
        lo_, hi_ = bounds[c], bounds[c + 1]
        np.add.at(cnt[c], (blk[lo_:hi_], q_of[lo_:hi_]), 1)
    k2 = (cnt.max(axis=0) + P - 1) // P            # [NBLK, NQ]
    cnt1 = cnt.sum(axis=2)                         # [NC, NBLK]
    k1 = np.maximum(1, (cnt1.max(axis=0) + P - 1) // P)  # [NBLK]
    cum1 = np.concatenate([[0], np.cumsum(k1)]).astype(int)
    nch1 = int(cum1[-1])

    # L2/L3 column layout: group-major, then piece, then block
    colstart = np.zeros((NBLK, NQ), dtype=np.int64)
    cstart = np.zeros((NG, NQ), dtype=np.int64)
    K = np.zeros((NG, NQ), dtype=np.int64)
    gstart = np.zeros(NG + 1, dtype=np.int64)
    pos = 0
    for g in range(NG):
        gstart[g] = pos
        for q in range(NQ):
            cstart[g, q] = pos
            for b in range(g * GSZ, (g + 1) * GSZ):
                colstart[b, q] = pos
                pos += k2[b, q]
            K[g, q] = pos - cstart[g, q]
    gstart[NG] = pos
    nch2 = int(pos)
    cols2 = [
        [int(colstart[b, q]) + i for q in range(NQ) for i in range(k2[b, q])]
        for b in range(NBLK)
    ]
    # block-major S-table column layout (for contiguous per-block S build)
    cp2 = k2.sum(axis=1)                        # chunks per block
    scum2 = np.concatenate([[0], np.cumsum(cp2)]).astype(int)
    # global gt column -> block-major S column
    s_of_col = np.zeros(nch2, dtype=np.int64)
    for b in range(NBLK):
        for ci, col in enumerate(cols2[b]):
            s_of_col[col] = scum2[b] + ci
    wstart = np.zeros((NG, NQ), dtype=np.int64)
    tw = 0
    for g in range(NG):
        for q in range(NQ):
            wstart[g, q] = tw
            tw += 8 * int(K[g, q])
    TW = int(tw)

    per_core = []
    for c in range(NC):
        lo_, hi_ = bounds[c], bounds[c + 1]
        b_c = blk[lo_:hi_]
        q_c = q_of[lo_:hi_]
        pr_c = prow[lo_:hi_]
        d_c = d_s[lo_:hi_] - c * SH
        s_c = s_s[lo_:hi_]

        # ---- L2/L3 slot assignment: order by (block, piece) ----
        key = b_c * NQ + q_c
        o2 = np.argsort(key, kind="stable")
        key_s = key[o2]
        # position within each (b,q) run
        starts = np.searchsorted(key_s, np.arange(NBLK * NQ))
        pos_in = np.arange(len(key_s)) - starts[key_s]
        chunk = pos_in >> 7
        part = pos_in & 127
        col = colstart.reshape(-1)[key_s] + chunk     # global chunk col

        scol = s_of_col[col]                    # block-major S column
        dstcol2 = np.full((P, nch2), -1, dtype=np.int16)
        dinvd2 = np.zeros((P, nch2), dtype=np.float32)  # cast to bf16 below
        dloc = d_c[o2]
        dstcol2[part, scol] = (dloc & 127).astype(np.int16)
        dinvd2[part, scol] = dinv[dloc + c * SH]

        # gather index table: wrapped-16 i16 layout per (g, q) region
        big = np.zeros(P * nch2, dtype=np.int16)
        big[col * P + part] = pr_c[o2].astype(np.int16)
        idx2 = np.zeros((P, TW), dtype=np.int16)
        for g in range(NG):
            for q in range(NQ):
                kk = int(K[g, q])
                if kk == 0:
                    continue
                arr = big[int(cstart[g, q]) * P:(int(cstart[g, q]) + kk) * P]
                wr = arr.reshape(-1, 16).T            # [16, 8*kk]
                ws = int(wstart[g, q])
                idx2[:, ws:ws + 8 * kk] = np.tile(wr, (8, 1))

        # ---- L1 slot assignment: order by block only ----
        o1 = np.argsort(b_c, kind="stable")
        b1s = b_c[o1]
        starts1 = np.searchsorted(b1s, np.arange(NBLK))
        pos1 = np.arange(len(b1s)) - starts1[b1s]
        chunk_1 = pos1 >> 7
        part1 = pos1 & 127
        col1 = cum1[b1s] + chunk_1

        xe = np.zeros((P, nch1, DIN), dtype=BF16)
        xe[part1, col1, :] = xs[s_c[o1]]
        dstcol1 = np.full((P, nch1), -1, dtype=np.int16)
        dinvd1 = np.zeros((P, nch1), dtype=np.float32)
        d1 = d_c[o1]
        dstcol1[part1, col1] = (d1 & 127).astype(np.int16)
        dinvd1[part1, col1] = dinv[d1 + c * SH]
        xe = xe.reshape(P, nch1 * DIN)

        # dinv of own shard in [p, w] window layout (phase-1 scaling)
        ids = c * SH + np.arange(NBLK * P)
        valid = ids < (c + 1) * SH
        dc_own = np.where(valid, dinv[np.minimum(ids, N - 1)], 0.0)
        dc_own = dc_own.reshape(NBLK, P).T.astype(np.float32).copy()

        per_core.append({
            "xe": xe,
            "dc1": dstcol1, "dv1": dinvd1.astype(BF16),
            "dc2": dstcol2, "dv2": dinvd2.astype(BF16),
            "idx2": idx2,
            "dco": dc_own,
        })

    meta = {
        "k2": k2.tolist(), "k1": k1.tolist(), "cum1": cum1.tolist(),
        "nch1": nch1, "nch2": nch2, "TW": TW,
        "cstart": cstart.tolist(), "K": K.tolist(),
        "gstart": gstart.tolist(), "wstart": wstart.tolist(),
        "cols2": cols2, "cp2": cp2.tolist(), "scum2": scum2.tolist(),
    }
    return per_core, meta, perm


def _build_program(meta):
    from concourse import bass, bacc, mybir
    import concourse.tile as tile

    f32 = mybir.dt.float32
    bf16 = mybir.dt.bfloat16
    i16 = mybir.dt.int16
    i32 = mybir.dt.int32
    k1 = meta["k1"]
    cum1 = meta["cum1"]
    nch1 = meta["nch1"]
    nch2 = meta["nch2"]
    TW = meta["TW"]
    wstart = meta["wstart"]
    cstart = meta["cstart"]
    K = meta["K"]
    gstart = meta["gstart"]
    cols2 = meta["cols2"]
    cp2 = meta["cp2"]
    scum2 = meta["scum2"]
    mxk1 = max(k1)
    mxcp = max(max(k1), max(cp2))
    mxgw = max(gstart[g + 1] - gstart[g] for g in range(NG))

    nc = bacc.Bacc("TRN2", target_bir_lowering=False, debug=False,
                   dynamic_dma_scratch_size=65536)

    xe = nc.declare_dram_parameter("xe", [P, nch1 * DIN], bf16, isOutput=False)
    dc1 = nc.declare_dram_parameter("dc1", [P, nch1], i16, isOutput=False)
    dv1 = nc.declare_dram_parameter("dv1", [P, nch1], bf16, isOutput=False)
    dc2 = nc.declare_dram_parameter("dc2", [P, nch2], i16, isOutput=False)
    dv2 = nc.declare_dram_parameter("dv2", [P, nch2], bf16, isOutput=False)
    idx2 = nc.declare_dram_parameter("idx2", [P, TW], i16, isOutput=False)
    dco = nc.declare_dram_parameter("dco", [P, NBLK], f32, isOutput=False)
    w1 = nc.declare_dram_parameter("w1", [P, DH], bf16, isOutput=False)
    w2p = nc.declare_dram_parameter("w2p", [P, 2 * DH], bf16, isOutput=False)
    w3p = nc.declare_dram_parameter("w3p", [P, 2 * DH], bf16, isOutput=False)
    bt = nc.declare_dram_parameter("bt", [P, 4], f32, isOutput=False)
    bf3 = nc.declare_dram_parameter("bf3", [P, DH], f32, isOutput=False)
    iot = nc.declare_dram_parameter("iot", [P, mxcp * P], i16, isOutput=False)
    outp = nc.declare_dram_parameter("out", [SH, DH], f32, isOutput=True)

    ybin2 = [nc.dram_tensor(f"ybin2_{q}", [RQ[q], DH], bf16) for q in range(NQ)]
    ybout2 = [nc.dram_tensor(f"ybout2_{q}", [NC * RQ[q], DH], bf16,
                             addr_space="Shared") for q in range(NQ)]
    ybin3 = [nc.dram_tensor(f"ybin3_{q}", [RQ[q], DH], bf16) for q in range(NQ)]
    ybout3 = [nc.dram_tensor(f"ybout3_{q}", [NC * RQ[q], DH], bf16,
                             addr_space="Shared") for q in range(NQ)]

    AG = mybir.AluOpType
    ACT = mybir.ActivationFunctionType

    def piece_of_window(w):
        for q in range(NQ):
            if AG_WB[q] <= w < AG_WB[q + 1]:
                return q
        raise AssertionError

    with tile.TileContext(nc, linearize=bool(os.environ.get("KLIN"))) as tc:
        with (
            tc.tile_pool(name="const", bufs=1) as cp_,
            tc.tile_pool(name="sb", bufs=2) as sb,
            tc.tile_pool(name="stp", bufs=2) as stp,
            tc.tile_pool(name="gp", bufs=2) as gp,
            tc.tile_pool(name="xb", bufs=2) as xbp,
            tc.tile_pool(name="pp", bufs=2, space="PSUM") as pp,
            tc.tile_pool(name="ph", bufs=6, space="PSUM") as ph,
        ):
            w1sb = cp_.tile([P, DH], dtype=bf16)
            nc.sync.dma_start(out=w1sb[:], in_=w1[:, :])
            w2sb = cp_.tile([P, 2 * DH], dtype=bf16)
            nc.sync.dma_start(out=w2sb[:], in_=w2p[:, :])
            w3sb = cp_.tile([P, 2 * DH], dtype=bf16)
            nc.sync.dma_start(out=w3sb[:], in_=w3p[:, :])
            btsb = cp_.tile([P, 4], dtype=f32)
            nc.sync.dma_start(out=btsb[:], in_=bt[:, :])
            bf3sb = cp_.tile([P, DH], dtype=f32)
            nc.sync.dma_start(out=bf3sb[:], in_=bf3[:, :])
            dcosb = cp_.tile([P, NBLK], dtype=f32)
            nc.sync.dma_start(out=dcosb[:], in_=dco[:, :])
            iotsb = cp_.tile([P, mxcp * P], dtype=i16)
            nc.sync.dma_start(out=iotsb[:], in_=iot[:, :])
            dc1sb = cp_.tile([P, nch1], dtype=i16)
            nc.sync.dma_start(out=dc1sb[:], in_=dc1[:, :])
            dv1sb = cp_.tile([P, nch1], dtype=bf16)
            nc.sync.dma_start(out=dv1sb[:], in_=dv1[:, :])
            dc2sb = cp_.tile([P, nch2], dtype=i16)
            nc.sync.dma_start(out=dc2sb[:], in_=dc2[:, :])
            dv2sb = cp_.tile([P, nch2], dtype=bf16)
            nc.sync.dma_start(out=dv2sb[:], in_=dv2[:, :])
            idxsb = cp_.tile([P, TW], dtype=i16)
            nc.sync.dma_start(out=idxsb[:], in_=idx2[:, :])
            # resident transposed activations h^T: half h at cols [h*SH, ...)
            xts = cp_.tile([P, 2 * SH], dtype=bf16)

            def sbuild_block(dcsb, dvsb, c0, cp):
                """All of a block's one-hot scatter chunks in two DVE ops:
                S[p, k*128+c] = (c == dstcol[p, c0+k]) * dinv[p, c0+k]."""
                eq = stp.tile([P, mxcp * P], dtype=bf16, tag="eq")
                nc.vector.tensor_tensor(
                    out=eq[:, :cp * P].rearrange("p (k e) -> p k e", e=P),
                    in0=iotsb[:, :cp * P].rearrange("p (k e) -> p k e", e=P),
                    in1=dcsb[:, c0:c0 + cp].to_broadcast((P, cp, P)),
                    op=AG.is_equal)
                st = stp.tile([P, mxcp * P], dtype=bf16, tag="st")
                nc.vector.tensor_tensor(
                    out=st[:, :cp * P].rearrange("p (k e) -> p k e", e=P),
                    in0=eq[:, :cp * P].rearrange("p (k e) -> p k e", e=P),
                    in1=dvsb[:, c0:c0 + cp].to_broadcast((P, cp, P)),
                    op=AG.mult)
                return st

            def phase1_win(wsb, ybinq, w):
                """One window of Y = dinv * (h @ W) from xts -> ybin rows."""
                m = LASTM if w == NBLK - 1 else P
                ps = pp.tile([P, DH], dtype=f32, tag="ps")
                for h in range(2):
                    nc.tensor.matmul(
                        out=ps[:m, :],
                        lhsT=xts[:, h * SH + w * P:h * SH + w * P + m],
                        rhs=wsb[:, h * DH:(h + 1) * DH],
                        start=(h == 0), stop=(h == 1))
                ysb = sb.tile([P, DH], dtype=bf16, tag="ysb")
                nc.scalar.activation(out=ysb[:m, :], in_=ps[:m, :],
                                     func=ACT.Copy,
                                     scale=dcosb[:m, w:w + 1])
                q = piece_of_window(w)
                r0 = w * P - AG_LO[q]
                nc.sync.dma_start(out=ybinq[q][r0:r0 + m, :], in_=ysb[:m, :])

            def all_gather_piece(ybinq, yboutq, q):
                nc.gpsimd.collective_compute(
                    "AllGather", AG.bypass,
                    replica_groups=[list(range(NC))],
                    ins=[ybinq[q][0:RQ[q], :].opt()],
                    outs=[yboutq[q][0:NC * RQ[q], :].opt()])

            gmax = int(os.environ.get("KGMAX", "8"))   # chunks per gather

            def gather_pieces(gt, g, tableq, pieces):
                """Issue <=gmax-chunk dma_gathers for the given pieces of
                group g into tile gt (the wrapped-16 idx layout slices at
                chunk granularity). NOTE: the gpsimd queue is in-order, so a
                gather must be issued AFTER its piece's AllGather or the
                queue-head wait deadlocks."""
                g0 = gstart[g]
                for q in pieces:
                    kk = K[g][q]
                    c0 = cstart[g][q] - g0      # column offset inside tile
                    ws = wstart[g][q]
                    for j0 in range(0, kk, gmax):
                        j1 = min(j0 + gmax, kk)
                        out_ap = gt[:, (c0 + j0) * DH:(c0 + j1) * DH].rearrange(
                            "p (k e) -> p k e", e=DH)
                        nc.gpsimd.dma_gather(
                            out_ap,
                            tableq[q][0:NC * RQ[q], :],
                            idxsb[:, ws + 8 * j0:ws + 8 * j1],
                            P * (j1 - j0),
                            P * (j1 - j0),
                            DH)
                return gt

            NHOIST = 2          # groups whose early-piece gathers are issued
                                # before the last AG piece, so the Pool engine
                                # works instead of idling behind it

            def hoist_gathers(tableq):
                """Pre-issue pieces 0..NQ-2 for the first NHOIST groups."""
                pre = {}
                for g in range(NHOIST):
                    gt = gp.tile([P, mxgw * DH], dtype=bf16, tag="gt")
                    gather_pieces(gt, g, tableq, list(range(NQ - 1)))
                    pre[g] = gt
                return pre

            def group_tile(pre, g, tableq):
                if g in pre:
                    gt = pre.pop(g)
                    gather_pieces(gt, g, tableq, [NQ - 1])
                else:
                    gt = gp.tile([P, mxgw * DH], dtype=bf16, tag="gt")
                    gather_pieces(gt, g, tableq, list(range(NQ)))
                return gt

            # ---------------- Layer 1: streamed edge table ------------------
            for b in range(NBLK):
                kb = k1[b]
                m = LASTM if b == NBLK - 1 else P
                xet = xbp.tile([P, mxk1 * DIN], dtype=bf16, tag="xet")
                nc.sync.dma_start(
                    out=xet[:, :kb * DIN],
                    in_=xe[:, cum1[b] * DIN:(cum1[b] + kb) * DIN])
                psa = ph.tile([P, P], dtype=f32, tag="half")
                stb = sbuild_block(dc1sb, dv1sb, cum1[b], kb)
                for i in range(kb):
                    nc.tensor.matmul(
                        out=psa[:, :m],
                        lhsT=xet[:, i * DIN:(i + 1) * DIN],
                        rhs=stb[:, i * P:i * P + m],
                        start=(i == 0), stop=(i == kb - 1))
                agg = sb.tile([P, P], dtype=bf16, tag="agg")
                nc.scalar.activation(out=agg[:, :m], in_=psa[:, :m],
                                     func=ACT.Copy)
                psb = [ph.tile([P, P], dtype=f32, tag="half", name=f"psb{h}")
                       for h in range(2)]
                for h in range(2):
                    nc.tensor.matmul(
                        out=psb[h][:, :m],
                        lhsT=w1sb[:, h * P:(h + 1) * P],
                        rhs=agg[:, :m],
                        start=True, stop=True)
                for h in range(2):
                    nc.scalar.activation(
                        out=xts[:, h * SH + b * P:h * SH + b * P + m],
                        in_=psb[h][:, :m],
                        func=ACT.Relu, bias=btsb[:, h:h + 1])
                phase1_win(w2sb, ybin2, b)
                if b + 1 in AG_WB[1:NQ]:
                    all_gather_piece(ybin2, ybout2, AG_WB.index(b + 1) - 1)
            pre2 = hoist_gathers(ybout2)
            all_gather_piece(ybin2, ybout2, NQ - 1)

            # ---------------- Layer 2: transposed scatter -------------------
            for g in range(NG):
                gt = group_tile(pre2, g, ybout2)
                g0 = gstart[g]
                for b in range(g * GSZ, (g + 1) * GSZ):
                    m = LASTM if b == NBLK - 1 else P
                    cols = cols2[b]
                    pst = [ph.tile([P, P], dtype=f32, tag="half",
                                   name=f"pst{h}") for h in range(2)]
                    stb = sbuild_block(dc2sb, dv2sb, scum2[b], len(cols))
                    for ci, col in enumerate(cols):
                        lp = col - g0
                        for h in range(2):
                            nc.tensor.matmul(
                                out=pst[h][:, :m],
                                lhsT=gt[:, lp * DH + h * P:lp * DH + (h + 1) * P],
                                rhs=stb[:, ci * P:ci * P + m],
                                start=(ci == 0), stop=(ci == len(cols) - 1))
                    for h in range(2):
                        nc.scalar.activation(
                            out=xts[:, h * SH + b * P:h * SH + b * P + m],
                            in_=pst[h][:, :m],
                            func=ACT.Relu, bias=btsb[:, 2 + h:2 + h + 1])
                    phase1_win(w3sb, ybin3, b)
                    if b + 1 in AG_WB[1:NQ]:
                        all_gather_piece(ybin3, ybout3, AG_WB.index(b + 1) - 1)
            pre3 = hoist_gathers(ybout3)
            all_gather_piece(ybin3, ybout3, NQ - 1)

            # ---------------- Layer 3: direct scatter -> out ----------------
            for g in range(NG):
                gt = group_tile(pre3, g, ybout3)
                g0 = gstart[g]
                for b in range(g * GSZ, (g + 1) * GSZ):
                    m = LASTM if b == NBLK - 1 else P
                    cols = cols2[b]
                    ps3 = pp.tile([P, DH], dtype=f32, tag="ps")
                    stb = sbuild_block(dc2sb, dv2sb, scum2[b], len(cols))
                    for ci, col in enumerate(cols):
                        lp = col - g0
                        nc.tensor.matmul(
                            out=ps3[:m, :],
                            lhsT=stb[:, ci * P:ci * P + m],
                            rhs=gt[:, lp * DH:(lp + 1) * DH],
                            start=(ci == 0), stop=(ci == len(cols) - 1))
                    osb = sb.tile([P, DH], dtype=f32, tag="osb")
                    nc.vector.tensor_tensor(out=osb[:m, :], in0=ps3[:m, :],
                                            in1=bf3sb[:m, :], op=AG.add)
                    nc.sync.dma_start(out=outp[b * P:b * P + m, :],
                                      in_=osb[:m, :])

    nc.compile()
    return nc


def kernel(x, edge_index, W1, b1, W2, b2, W3, b3, _trace=False):
    from concourse.bass_utils import run_bass_kernel_spmd

    x = np.asarray(x, dtype=np.float32)
    per_core, meta, perm = _preprocess(x, edge_index)
    nc = _build_program(meta)

    w2 = np.asarray(W2, np.float32)
    w3 = np.asarray(W3, np.float32)
    w2p = np.concatenate([w2[0:P, :], w2[P:2 * P, :]], axis=1).astype(BF16)
    w3p = np.concatenate([w3[0:P, :], w3[P:2 * P, :]], axis=1).astype(BF16)
    b1v = np.asarray(b1, np.float32)
    b2v = np.asarray(b2, np.float32)
    bt = np.stack([b1v[0:P], b1v[P:2 * P], b2v[0:P], b2v[P:2 * P]],
                  axis=1).astype(np.float32)
    common = {
        "w1": np.asarray(W1, np.float32).astype(BF16),
        "w2p": w2p,
        "w3p": w3p,
        "bt": bt,
        "bf3": np.broadcast_to(np.asarray(b3, np.float32), (P, DH)).copy(),
    }
    mxcp = max(max(meta["k1"]), max(meta["cp2"]))
    common["iot"] = np.broadcast_to(
        np.tile(np.arange(P, dtype=np.int16), mxcp), (P, mxcp * P)).copy()
    in_maps = []
    for c in range(NC):
        m = dict(common)
        m.update(per_core[c])
        in_maps.append(m)

    res = run_bass_kernel_spmd(nc, in_maps, list(range(NC)), trace=_trace)
    shards = [res.results[c]["out"] for c in range(NC)]
    out = np.concatenate(shards, axis=0)[perm]
    if _trace:
        return out, res
    return out


# revision 37
# speedup vs baseline: 1.0404x; 1.0121x over previous
"""3-layer GCN (message passing) on 8 NeuronCores via Bass/Tile.

Strategy (vertex-cut / dst-sharding, bf16 data path):
  - Nodes are LPT-packed into (core, block) bins by in-degree so every
    128-dst block needs a similar number of edge chunks on every core
    (SPMD program). Output rows are un-permuted on the host at the end.
  - Self loops are regular edges (src=dst), no special casing.
  - Layer 1 is re-associated: relu(A_hat (X W1) + b1) = relu((A_hat X') W1
    + b1) with X' = dinv*X pre-scaled on host and expanded to edge order
    (xe), so L1 needs no indirect gathers at all - pure streaming.
  - Layers 2/3: each core computes its Y = dinv*(h @ W) shard; the table
    is AllGathered in FOUR row-pieces (separate DRAM tensors so the tile
    dep tracker gives piece-granular ordering), interleaved with the
    producing loop. Edges are grouped by (7-block group, source piece)
    and fetched with batched gpsimd dma_gathers of <=1024 rows (=64
    descriptors/SDMA engine, the packet ceiling; larger launches wedge
    the device). Measured SWDGE cost ~1us fixed + ~7.4ns/row of Q7
    descriptor generation - that per-row cost is the kernel's floor.
  - Scatter-add realized on TensorE with one-hot scatter matrices
    S[e, dst] = dinv[dst] built ON-CHIP per BLOCK by two broadcast
    VectorE tensor_tensors: (iota == dstcol bcast) * dinvd bcast
    (per-chunk tensor_scalar was ~716ns/op on DVE and jammed it).
  - L1/L2 scatter runs transposed (psT = G^T @ S) so the ReLU epilogue
    writes h^T directly into the resident xts tile; bias+relu fused into
    one ScalarE activation per feature half. The next layer's phase-1
    window is interleaved after each block.
  - L3 scatter runs direct (ps = S^T @ G) to emit [dst, feat] fp32 rows.
"""

import os
import sys

sys.path.insert(0, "/opt/trn_rl_repo")

import numpy as np
import ml_dtypes

BF16 = ml_dtypes.bfloat16

N = 50000
E = 500000
NC = 8
SH = N // NC            # 6250 nodes per core
P = 128
DIN = 128
DH = 256
NBLK = (SH + P - 1) // P      # 49 dst blocks per core
LASTM = SH - (NBLK - 1) * P   # 106 dsts in the last block
GSZ = 7                       # blocks per gather group
NG = NBLK // GSZ              # 7 groups
NQ = 4                        # AllGather pieces
AG_WB = [0, 13, 25, 37, NBLK]            # window boundaries per AG piece
AG_LO = [w * P for w in AG_WB[:NQ]]       # piece start rows (per core)
RQ = [min(AG_WB[q + 1] * P, SH) - AG_LO[q] for q in range(NQ)]  # piece rows


def _balance(deg):
    """LPT-pack nodes into (core, block) bins by in-edge weight so every
    block's edge count is ~equal across cores. Returns perm: node -> device
    row (core*SH + block*128 + slot)."""
    import heapq
    w = deg.astype(np.int64)
    nodes = np.argsort(-w, kind="stable")
    caps = []
    for c in range(NC):
        for b in range(NBLK):
            cap = LASTM if b == NBLK - 1 else P
            caps.append((c * SH + b * P, cap))
    heap = [(0, i) for i in range(len(caps))]
    heapq.heapify(heap)
    fill = [0] * len(caps)
    perm = np.empty(N, dtype=np.int64)
    for n in nodes:
        while True:
            wt, i = heapq.heappop(heap)
            if fill[i] < caps[i][1]:
                break
        perm[n] = caps[i][0] + fill[i]
        fill[i] += 1
        if fill[i] < caps[i][1]:
            heapq.heappush(heap, (wt + int(w[n]), i))
    return perm


def _layout():
    """Static (data-independent) meta helpers."""
    pass


def _preprocess(x, edge_index):
    """Host-side graph partitioning. Returns per-core tensors + layout meta."""
    src = np.asarray(edge_index[0], dtype=np.int64)
    dst = np.asarray(edge_index[1], dtype=np.int64)
    deg = np.bincount(dst, minlength=N).astype(np.float64)
    dinv0 = (1.0 / np.sqrt(deg + 1.0)).astype(np.float32)

    perm = _balance(deg)
    inv = np.empty(N, dtype=np.int64)
    inv[perm] = np.arange(N)           # device row -> original node
    src = perm[src]
    dst = perm[dst]
    x = np.asarray(x, np.float32)[inv]
    dinv = dinv0[inv]

    # self loops as regular edges
    loop = np.arange(N, dtype=np.int64)
    src = np.concatenate([src, loop])
    dst = np.concatenate([dst, loop])

    order = np.argsort(dst, kind="stable")
    s_s = src[order]
    d_s = dst[order]
    bounds = np.searchsorted(d_s, np.arange(NC + 1) * SH)

    xs = (dinv[:, None] * x).astype(BF16)  # [N, 128] pre-scaled features

    # source piece + piece-local table row for every edge
    gc = s_s // SH
    gr = s_s % SH
    q_of = np.searchsorted(np.array(AG_LO[1:]), gr, side="right")  # [TE]
    rq = np.array(RQ)[q_of]
    lo = np.array(AG_LO)[q_of]
    prow = gc * rq + (gr - lo)         # row within piece-q gathered table
    blk = np.minimum((d_s % SH) >> 7, NBLK - 1)

    # per-core per-(block, piece) counts -> shared chunk counts k2
    cnt = np.zeros((NC, NBLK, NQ), dtype=np.int64)
    for c in range(NC):
        lo_, hi_ = bounds[c], bounds[c + 1]
        np.add.at(cnt[c], (blk[lo_:hi_], q_of[lo_:hi_]), 1)
    k2 = (cnt.max(axis=0) + P - 1) // P            # [NBLK, NQ]
    cnt1 = cnt.sum(axis=2)                         # [NC, NBLK]
    k1 = np.maximum(1, (cnt1.max(axis=0) + P - 1) // P)  # [NBLK]
    cum1 = np.concatenate([[0], np.cumsum(k1)]).astype(int)
    nch1 = int(cum1[-1])

    # L2/L3 column layout: group-major, then piece, then block
    colstart = np.zeros((NBLK, NQ), dtype=np.int64)
    cstart = np.zeros((NG, NQ), dtype=np.int64)
    K = np.zeros((NG, NQ), dtype=np.int64)
    gstart = np.zeros(NG + 1, dtype=np.int64)
    pos = 0
    for g in range(NG):
        gstart[g] = pos
        for q in range(NQ):
            cstart[g, q] = pos
            for b in range(g * GSZ, (g + 1) * GSZ):
                colstart[b, q] = pos
                pos += k2[b, q]
            K[g, q] = pos - cstart[g, q]
    gstart[NG] = pos
    nch2 = int(pos)
    cols2 = [
        [int(colstart[b, q]) + i for q in range(NQ) for i in range(k2[b, q])]
        for b in range(NBLK)
    ]
    # block-major S-table column layout (for contiguous per-block S build)
    cp2 = k2.sum(axis=1)                        # chunks per block
    scum2 = np.concatenate([[0], np.cumsum(cp2)]).astype(int)
    # global gt column -> block-major S column
    s_of_col = np.zeros(nch2, dtype=np.int64)
    for b in range(NBLK):
        for ci, col in enumerate(cols2[b]):
            s_of_col[col] = scum2[b] + ci
    wstart = np.zeros((NG, NQ), dtype=np.int64)
    tw = 0
    for g in range(NG):
        for q in range(NQ):
            wstart[g, q] = tw
            tw += 8 * int(K[g, q])
    TW = int(tw)

    per_core = []
    for c in range(NC):
        lo_, hi_ = bounds[c], bounds[c + 1]
        b_c = blk[lo_:hi_]
        q_c = q_of[lo_:hi_]
        pr_c = prow[lo_:hi_]
        d_c = d_s[lo_:hi_] - c * SH
        s_c = s_s[lo_:hi_]

        # ---- L2/L3 slot assignment: order by (block, piece) ----
        key = b_c * NQ + q_c
        o2 = np.argsort(key, kind="stable")
        key_s = key[o2]
        # position within each (b,q) run
        starts = np.searchsorted(key_s, np.arange(NBLK * NQ))
        pos_in = np.arange(len(key_s)) - starts[key_s]
        chunk = pos_in >> 7
        part = pos_in & 127
        col = colstart.reshape(-1)[key_s] + chunk     # global chunk col

        scol = s_of_col[col]                    # block-major S column
        dstcol2 = np.full((P, nch2), -1, dtype=np.int16)
        dinvd2 = np.zeros((P, nch2), dtype=np.float32)  # cast to bf16 below
        dloc = d_c[o2]
        dstcol2[part, scol] = (dloc & 127).astype(np.int16)
        dinvd2[part, scol] = dinv[dloc + c * SH]

        # gather index table: wrapped-16 i16 layout per (g, q) region
        big = np.zeros(P * nch2, dtype=np.int16)
        big[col * P + part] = pr_c[o2].astype(np.int16)
        idx2 = np.zeros((P, TW), dtype=np.int16)
        for g in range(NG):
            for q in range(NQ):
                kk = int(K[g, q])
                if kk == 0:
                    continue
                arr = big[int(cstart[g, q]) * P:(int(cstart[g, q]) + kk) * P]
                wr = arr.reshape(-1, 16).T            # [16, 8*kk]
                ws = int(wstart[g, q])
                idx2[:, ws:ws + 8 * kk] = np.tile(wr, (8, 1))

        # ---- L1 slot assignment: order by block only ----
        o1 = np.argsort(b_c, kind="stable")
        b1s = b_c[o1]
        starts1 = np.searchsorted(b1s, np.arange(NBLK))
        pos1 = np.arange(len(b1s)) - starts1[b1s]
        chunk_1 = pos1 >> 7
        part1 = pos1 & 127
        col1 = cum1[b1s] + chunk_1

        xe = np.zeros((P, nch1, DIN), dtype=BF16)
        xe[part1, col1, :] = xs[s_c[o1]]
        dstcol1 = np.full((P, nch1), -1, dtype=np.int16)
        dinvd1 = np.zeros((P, nch1), dtype=np.float32)
        d1 = d_c[o1]
        dstcol1[part1, col1] = (d1 & 127).astype(np.int16)
        dinvd1[part1, col1] = dinv[d1 + c * SH]
        xe = xe.reshape(P, nch1 * DIN)

        # dinv of own shard in [p, w] window layout (phase-1 scaling)
        ids = c * SH + np.arange(NBLK * P)
        valid = ids < (c + 1) * SH
        dc_own = np.where(valid, dinv[np.minimum(ids, N - 1)], 0.0)
        dc_own = dc_own.reshape(NBLK, P).T.astype(np.float32).copy()

        per_core.append({
            "xe": xe,
            "dc1": dstcol1, "dv1": dinvd1.astype(BF16),
            "dc2": dstcol2, "dv2": dinvd2.astype(BF16),
            "idx2": idx2,
            "dco": dc_own,
        })

    meta = {
        "k2": k2.tolist(), "k1": k1.tolist(), "cum1": cum1.tolist(),
        "nch1": nch1, "nch2": nch2, "TW": TW,
        "cstart": cstart.tolist(), "K": K.tolist(),
        "gstart": gstart.tolist(), "wstart": wstart.tolist(),
        "cols2": cols2, "cp2": cp2.tolist(), "scum2": scum2.tolist(),
    }
    return per_core, meta, perm


def _build_program(meta):
    from concourse import bass, bacc, mybir
    import concourse.tile as tile

    f32 = mybir.dt.float32
    bf16 = mybir.dt.bfloat16
    i16 = mybir.dt.int16
    i32 = mybir.dt.int32
    k1 = meta["k1"]
    cum1 = meta["cum1"]
    nch1 = meta["nch1"]
    nch2 = meta["nch2"]
    TW = meta["TW"]
    wstart = meta["wstart"]
    cstart = meta["cstart"]
    K = meta["K"]
    gstart = meta["gstart"]
    cols2 = meta["cols2"]
    cp2 = meta["cp2"]
    scum2 = meta["scum2"]
    mxk1 = max(k1)
    mxcp = max(max(k1), max(cp2))
    mxgw = max(gstart[g + 1] - gstart[g] for g in range(NG))

    nc = bacc.Bacc("TRN2", target_bir_lowering=False, debug=False,
                   dynamic_dma_scratch_size=65536)

    xe = nc.declare_dram_parameter("xe", [P, nch1 * DIN], bf16, isOutput=False)
    dc1 = nc.declare_dram_parameter("dc1", [P, nch1], i16, isOutput=False)
    dv1 = nc.declare_dram_parameter("dv1", [P, nch1], bf16, isOutput=False)
    dc2 = nc.declare_dram_parameter("dc2", [P, nch2], i16, isOutput=False)
    dv2 = nc.declare_dram_parameter("dv2", [P, nch2], bf16, isOutput=False)
    idx2 = nc.declare_dram_parameter("idx2", [P, TW], i16, isOutput=False)
    dco = nc.declare_dram_parameter("dco", [P, NBLK], f32, isOutput=False)
    w1 = nc.declare_dram_parameter("w1", [P, DH], bf16, isOutput=False)
    w2p = nc.declare_dram_parameter("w2p", [P, 2 * DH], bf16, isOutput=False)
    w3p = nc.declare_dram_parameter("w3p", [P, 2 * DH], bf16, isOutput=False)
    bt = nc.declare_dram_parameter("bt", [P, 4], f32, isOutput=False)
    bf3 = nc.declare_dram_parameter("bf3", [P, DH], f32, isOutput=False)
    iot = nc.declare_dram_parameter("iot", [P, mxcp * P], i16, isOutput=False)
    outp = nc.declare_dram_parameter("out", [SH, DH], f32, isOutput=True)

    ybin2 = [nc.dram_tensor(f"ybin2_{q}", [RQ[q], DH], bf16) for q in range(NQ)]
    ybout2 = [nc.dram_tensor(f"ybout2_{q}", [NC * RQ[q], DH], bf16,
                             addr_space="Shared") for q in range(NQ)]
    ybin3 = [nc.dram_tensor(f"ybin3_{q}", [RQ[q], DH], bf16) for q in range(NQ)]
    ybout3 = [nc.dram_tensor(f"ybout3_{q}", [NC * RQ[q], DH], bf16,
                             addr_space="Shared") for q in range(NQ)]

    AG = mybir.AluOpType
    ACT = mybir.ActivationFunctionType

    def piece_of_window(w):
        for q in range(NQ):
            if AG_WB[q] <= w < AG_WB[q + 1]:
                return q
        raise AssertionError

    with tile.TileContext(nc, linearize=bool(os.environ.get("KLIN"))) as tc:
        with (
            tc.tile_pool(name="const", bufs=1) as cp_,
            tc.tile_pool(name="sb", bufs=2) as sb,
            tc.tile_pool(name="stp", bufs=2) as stp,
            tc.tile_pool(name="gp", bufs=2) as gp,
            tc.tile_pool(name="xb", bufs=2) as xbp,
            tc.tile_pool(name="pp", bufs=2, space="PSUM") as pp,
            tc.tile_pool(name="ph", bufs=6, space="PSUM") as ph,
        ):
            w1sb = cp_.tile([P, DH], dtype=bf16)
            nc.sync.dma_start(out=w1sb[:], in_=w1[:, :])
            w2sb = cp_.tile([P, 2 * DH], dtype=bf16)
            nc.sync.dma_start(out=w2sb[:], in_=w2p[:, :])
            w3sb = cp_.tile([P, 2 * DH], dtype=bf16)
            nc.sync.dma_start(out=w3sb[:], in_=w3p[:, :])
            btsb = cp_.tile([P, 4], dtype=f32)
            nc.sync.dma_start(out=btsb[:], in_=bt[:, :])
            bf3sb = cp_.tile([P, DH], dtype=f32)
            nc.sync.dma_start(out=bf3sb[:], in_=bf3[:, :])
            dcosb = cp_.tile([P, NBLK], dtype=f32)
            nc.sync.dma_start(out=dcosb[:], in_=dco[:, :])
            iotsb = cp_.tile([P, mxcp * P], dtype=i16)
            nc.sync.dma_start(out=iotsb[:], in_=iot[:, :])
            dc1sb = cp_.tile([P, nch1], dtype=i16)
            nc.sync.dma_start(out=dc1sb[:], in_=dc1[:, :])
            dv1sb = cp_.tile([P, nch1], dtype=bf16)
            nc.sync.dma_start(out=dv1sb[:], in_=dv1[:, :])
            dc2sb = cp_.tile([P, nch2], dtype=i16)
            nc.sync.dma_start(out=dc2sb[:], in_=dc2[:, :])
            dv2sb = cp_.tile([P, nch2], dtype=bf16)
            nc.sync.dma_start(out=dv2sb[:], in_=dv2[:, :])
            idxsb = cp_.tile([P, TW], dtype=i16)
            nc.sync.dma_start(out=idxsb[:], in_=idx2[:, :])
            # resident transposed activations h^T: half h at cols [h*SH, ...)
            xts = cp_.tile([P, 2 * SH], dtype=bf16)

            def sbuild_block(dcsb, dvsb, c0, cp):
                """All of a block's one-hot scatter chunks in two DVE ops:
                S[p, k*128+c] = (c == dstcol[p, c0+k]) * dinv[p, c0+k]."""
                eq = stp.tile([P, mxcp * P], dtype=bf16, tag="eq")
                nc.vector.tensor_tensor(
                    out=eq[:, :cp * P].rearrange("p (k e) -> p k e", e=P),
                    in0=iotsb[:, :cp * P].rearrange("p (k e) -> p k e", e=P),
                    in1=dcsb[:, c0:c0 + cp].to_broadcast((P, cp, P)),
                    op=AG.is_equal)
                st = stp.tile([P, mxcp * P], dtype=bf16, tag="st")
                nc.vector.tensor_tensor(
                    out=st[:, :cp * P].rearrange("p (k e) -> p k e", e=P),
                    in0=eq[:, :cp * P].rearrange("p (k e) -> p k e", e=P),
                    in1=dvsb[:, c0:c0 + cp].to_broadcast((P, cp, P)),
                    op=AG.mult)
                return st

            def phase1_win(wsb, ybinq, w):
                """One window of Y = dinv * (h @ W) from xts -> ybin rows."""
                m = LASTM if w == NBLK - 1 else P
                ps = pp.tile([P, DH], dtype=f32, tag="ps")
                for h in range(2):
                    nc.tensor.matmul(
                        out=ps[:m, :],
                        lhsT=xts[:, h * SH + w * P:h * SH + w * P + m],
                        rhs=wsb[:, h * DH:(h + 1) * DH],
                        start=(h == 0), stop=(h == 1))
                ysb = sb.tile([P, DH], dtype=bf16, tag="ysb")
                nc.scalar.activation(out=ysb[:m, :], in_=ps[:m, :],
                                     func=ACT.Copy,
                                     scale=dcosb[:m, w:w + 1])
                q = piece_of_window(w)
                r0 = w * P - AG_LO[q]
                nc.sync.dma_start(out=ybinq[q][r0:r0 + m, :], in_=ysb[:m, :])

            def all_gather_piece(ybinq, yboutq, q):
                nc.gpsimd.collective_compute(
                    "AllGather", AG.bypass,
                    replica_groups=[list(range(NC))],
                    ins=[ybinq[q][0:RQ[q], :].opt()],
                    outs=[yboutq[q][0:NC * RQ[q], :].opt()])

            gmax = int(os.environ.get("KGMAX", "8"))   # chunks per gather

            def gather_pieces(gt, g, tableq, pieces):
                """Issue <=gmax-chunk dma_gathers for the given pieces of
                group g into tile gt (the wrapped-16 idx layout slices at
                chunk granularity). NOTE: the gpsimd queue is in-order, so a
                gather must be issued AFTER its piece's AllGather or the
                queue-head wait deadlocks."""
                g0 = gstart[g]
                for q in pieces:
                    kk = K[g][q]
                    c0 = cstart[g][q] - g0      # column offset inside tile
                    ws = wstart[g][q]
                    for j0 in range(0, kk, gmax):
                        j1 = min(j0 + gmax, kk)
                        out_ap = gt[:, (c0 + j0) * DH:(c0 + j1) * DH].rearrange(
                            "p (k e) -> p k e", e=DH)
                        nc.gpsimd.dma_gather(
                            out_ap,
                            tableq[q][0:NC * RQ[q], :],
                            idxsb[:, ws + 8 * j0:ws + 8 * j1],
                            P * (j1 - j0),
                            P * (j1 - j0),
                            DH)
                return gt

            NHOIST = 2          # groups whose early-piece gathers are issued
                                # before the last AG piece, so the Pool engine
                                # works instead of idling behind it

            def hoist_gathers(tableq):
                """Pre-issue pieces 0..NQ-2 for the first NHOIST groups."""
                pre = {}
                for g in range(NHOIST):
                    gt = gp.tile([P, mxgw * DH], dtype=bf16, tag="gt")
                    gather_pieces(gt, g, tableq, list(range(NQ - 1)))
                    pre[g] = gt
                return pre

            def group_tile(pre, g, tableq):
                if g in pre:
                    gt = pre.pop(g)
                    gather_pieces(gt, g, tableq, [NQ - 1])
                else:
                    gt = gp.tile([P, mxgw * DH], dtype=bf16, tag="gt")
                    gather_pieces(gt, g, tableq, list(range(NQ)))
                return gt

            # ---------------- Layer 1: streamed edge table ------------------
            # L2's early-piece gathers are issued inside this loop right
            # after each AG piece fires, so the Pool engine works during
            # the (otherwise gather-free) L1 prefix. Tiles pre-allocated.
            pre2 = {}
            for g in range(NHOIST):
                pre2[g] = gp.tile([P, mxgw * DH], dtype=bf16, tag="gt",
                                  name=f"pre2_{g}")
            for b in range(NBLK):
                kb = k1[b]
                m = LASTM if b == NBLK - 1 else P
                xet = xbp.tile([P, mxk1 * DIN], dtype=bf16, tag="xet")
                nc.sync.dma_start(
                    out=xet[:, :kb * DIN],
                    in_=xe[:, cum1[b] * DIN:(cum1[b] + kb) * DIN])
                psa = ph.tile([P, P], dtype=f32, tag="half")
                stb = sbuild_block(dc1sb, dv1sb, cum1[b], kb)
                for i in range(kb):
                    nc.tensor.matmul(
                        out=psa[:, :m],
                        lhsT=xet[:, i * DIN:(i + 1) * DIN],
                        rhs=stb[:, i * P:i * P + m],
                        start=(i == 0), stop=(i == kb - 1))
                agg = sb.tile([P, P], dtype=bf16, tag="agg")
                nc.scalar.activation(out=agg[:, :m], in_=psa[:, :m],
                                     func=ACT.Copy)
                psb = [ph.tile([P, P], dtype=f32, tag="half", name=f"psb{h}")
                       for h in range(2)]
                for h in range(2):
                    nc.tensor.matmul(
                        out=psb[h][:, :m],
                        lhsT=w1sb[:, h * P:(h + 1) * P],
                        rhs=agg[:, :m],
                        start=True, stop=True)
                for h in range(2):
                    nc.scalar.activation(
                        out=xts[:, h * SH + b * P:h * SH + b * P + m],
                        in_=psb[h][:, :m],
                        func=ACT.Relu, bias=btsb[:, h:h + 1])
                phase1_win(w2sb, ybin2, b)
                if b + 1 in AG_WB[1:NQ]:
                    qi = AG_WB.index(b + 1) - 1
                    all_gather_piece(ybin2, ybout2, qi)
                    for g in range(NHOIST):
                        gather_pieces(pre2[g], g, ybout2, [qi])
            all_gather_piece(ybin2, ybout2, NQ - 1)

            # ---------------- Layer 2: transposed scatter -------------------
            for g in range(NG):
                gt = group_tile(pre2, g, ybout2)
                g0 = gstart[g]
                for b in range(g * GSZ, (g + 1) * GSZ):
                    m = LASTM if b == NBLK - 1 else P
                    cols = cols2[b]
                    pst = [ph.tile([P, P], dtype=f32, tag="half",
                                   name=f"pst{h}") for h in range(2)]
                    stb = sbuild_block(dc2sb, dv2sb, scum2[b], len(cols))
                    for ci, col in enumerate(cols):
                        lp = col - g0
                        for h in range(2):
                            nc.tensor.matmul(
                                out=pst[h][:, :m],
                                lhsT=gt[:, lp * DH + h * P:lp * DH + (h + 1) * P],
                                rhs=stb[:, ci * P:ci * P + m],
                                start=(ci == 0), stop=(ci == len(cols) - 1))
                    for h in range(2):
                        nc.scalar.activation(
                            out=xts[:, h * SH + b * P:h * SH + b * P + m],
                            in_=pst[h][:, :m],
                            func=ACT.Relu, bias=btsb[:, 2 + h:2 + h + 1])
                    phase1_win(w3sb, ybin3, b)
                    if b + 1 in AG_WB[1:NQ]:
                        all_gather_piece(ybin3, ybout3, AG_WB.index(b + 1) - 1)
            pre3 = hoist_gathers(ybout3)
            all_gather_piece(ybin3, ybout3, NQ - 1)

            # ---------------- Layer 3: direct scatter -> out ----------------
            for g in range(NG):
                gt = group_tile(pre3, g, ybout3)
                g0 = gstart[g]
                for b in range(g * GSZ, (g + 1) * GSZ):
                    m = LASTM if b == NBLK - 1 else P
                    cols = cols2[b]
                    ps3 = pp.tile([P, DH], dtype=f32, tag="ps")
                    stb = sbuild_block(dc2sb, dv2sb, scum2[b], len(cols))
                    for ci, col in enumerate(cols):
                        lp = col - g0
                        nc.tensor.matmul(
                            out=ps3[:m, :],
                            lhsT=stb[:, ci * P:ci * P + m],
                            rhs=gt[:, lp * DH:(lp + 1) * DH],
                            start=(ci == 0), stop=(ci == len(cols) - 1))
                    osb = sb.tile([P, DH], dtype=f32, tag="osb")
                    nc.vector.tensor_tensor(out=osb[:m, :], in0=ps3[:m, :],
                                            in1=bf3sb[:m, :], op=AG.add)
                    nc.sync.dma_start(out=outp[b * P:b * P + m, :],
                                      in_=osb[:m, :])

    nc.compile()
    return nc


def kernel(x, edge_index, W1, b1, W2, b2, W3, b3, _trace=False):
    from concourse.bass_utils import run_bass_kernel_spmd

    x = np.asarray(x, dtype=np.float32)
    per_core, meta, perm = _preprocess(x, edge_index)
    nc = _build_program(meta)

    w2 = np.asarray(W2, np.float32)
    w3 = np.asarray(W3, np.float32)
    w2p = np.concatenate([w2[0:P, :], w2[P:2 * P, :]], axis=1).astype(BF16)
    w3p = np.concatenate([w3[0:P, :], w3[P:2 * P, :]], axis=1).astype(BF16)
    b1v = np.asarray(b1, np.float32)
    b2v = np.asarray(b2, np.float32)
    bt = np.stack([b1v[0:P], b1v[P:2 * P], b2v[0:P], b2v[P:2 * P]],
                  axis=1).astype(np.float32)
    common = {
        "w1": np.asarray(W1, np.float32).astype(BF16),
        "w2p": w2p,
        "w3p": w3p,
        "bt": bt,
        "bf3": np.broadcast_to(np.asarray(b3, np.float32), (P, DH)).copy(),
    }
    mxcp = max(max(meta["k1"]), max(meta["cp2"]))
    common["iot"] = np.broadcast_to(
        np.tile(np.arange(P, dtype=np.int16), mxcp), (P, mxcp * P)).copy()
    in_maps = []
    for c in range(NC):
        m = dict(common)
        m.update(per_core[c])
        in_maps.append(m)

    res = run_bass_kernel_spmd(nc, in_maps, list(range(NC)), trace=_trace)
    shards = [res.results[c]["out"] for c in range(NC)]
    out = np.concatenate(shards, axis=0)[perm]
    if _trace:
        return out, res
    return out
